# revision 1
# baseline (speedup 1.0000x reference)
"""Trainium2 Bass kernel for ContextAttentionMaskLuong.

Reference computation (per batch b):
    keys  = x @ W                       [B,S,D]
    query = tanh(c @ Wc + b)            [B,D]
    eij   = scale * <query, keys_s>     [B,S]
    a     = exp(eij - max) * mask; a /= (sum(a) + 1e-7)
    out   = sum_s a[s] * x[s,:]         [B,D]

Key rewrite: eij[b,s] = <x[b,s,:], q2[b]> with q2[b] = W @ query[b]
(i.e. q2 = query @ W^T), which removes the [B,S,D]x[D,D] matmul entirely.
The kernel is then one streaming pass over x (memory-bound).

Sharding: data-parallel over batch: 16 batches / 8 cores = 2 per core.
W/Wc/b/scale replicated.

Per-core layout (s-major): x tile t of batch b is SBUF [128, 4096] where
partition p, free q*1024+d  <->  x[b, 512*t + 4*p + q, d].
- eij via DVE tensor_tensor_reduce (fused mult+reduce along free dim)
- softmax via DVE reduce + gpsimd partition_all_reduce + ACT exp
- out via PE matmuls: lhsT = a-column [128,1], rhs = x slice [128,512]
"""

import numpy as np

B, S, D = 16, 2048, 1024
NCORES = 8
BPC = B // NCORES  # batches per core
EPS = 1e-7

TS = 4  # x tiles per batch
QT = 4  # s-rows per partition per tile
XF = QT * D  # x tile free size (4096)
SBLK = S // TS  # s-block per tile (512)

_CACHE = {}


def _build():
    import os

    phase = int(os.environ.get("KPHASE", "5"))
    repeat = int(os.environ.get("KREPEAT", "1"))
    import concourse.bass as bass
    import concourse.mybir as mybir
    import concourse.tile as tile
    from concourse import bacc
    from concourse import bass_isa
    from concourse.masks import make_identity

    fp32 = mybir.dt.float32
    i32 = mybir.dt.int32
    AF = mybir.ActivationFunctionType
    OP = mybir.AluOpType
    ts = bass.ts

    nc = bacc.Bacc(None)

    x_d = nc.dram_tensor("x", [BPC, S, D], fp32, kind="ExternalInput")
    mask_d = nc.dram_tensor("mask", [BPC, S], i32, kind="ExternalInput")
    c_d = nc.dram_tensor("c", [BPC, D], fp32, kind="ExternalInput")
    w_d = nc.dram_tensor("W", [D, D], fp32, kind="ExternalInput")
    wc_d = nc.dram_tensor("Wc", [D, D], fp32, kind="ExternalInput")
    b_d = nc.dram_tensor("b", [D], fp32, kind="ExternalInput")
    scale_d = nc.dram_tensor("scale", [1], fp32, kind="ExternalInput")
    out_d = nc.dram_tensor("out", [BPC, D], fp32, kind="ExternalOutput")

    KD = D // 128  # 8 chunks of 128 along d/e/c

    with tile.TileContext(nc) as tc:
        with (
            tc.tile_pool(name="const", bufs=1) as const,
            tc.tile_pool(name="xp", bufs=2 * TS) as xp,
            tc.tile_pool(name="persist", bufs=1) as persist,
            tc.tile_pool(name="scratch", bufs=2) as scratch,
            tc.tile_pool(name="psum", bufs=2, space="PSUM") as pp,
        ):
            # ---------- constants / small loads ----------
            identity = const.tile([128, 128], fp32, tag="identity")
            make_identity(nc, identity)
            ones1 = const.tile([1, 128], fp32, tag="ones1")
            nc.vector.memset(ones1, 1.0)
            ones_col = const.tile([128, 1], fp32, tag="ones_col")
            nc.vector.memset(ones_col, 1.0)

            scale_sb = const.tile([1, 1], fp32, tag="scale")
            nc.sync.dma_start(out=scale_sb, in_=scale_d[None, :])

            # bias transposed to e-on-partitions, replicated for both batches
            biasT = const.tile([128, KD, BPC], fp32, tag="biasT")
            for b in range(BPC):
                nc.sync.dma_start(
                    out=biasT[:, :, b], in_=b_d.rearrange("(k p) -> p k", p=128)
                )

            for _rep in range(repeat):
                # c transposed: cT[p, b, k] = c[b, 128k+p]  (single DMA)
                cT = const.tile([128, BPC, KD], fp32, tag="cT")
                nc.sync.dma_start(
                    out=cT, in_=c_d.rearrange("b (k p) -> p b k", p=128)
                )

                # mask (cast int32 -> f32 during DMA), layout matches eij cols
                mask_f = []
                for b in range(BPC):
                    mf = persist.tile([128, TS, QT], fp32, tag=f"mask{b}")
                    nc.gpsimd.dma_start(
                        out=mf,
                        in_=mask_d[b].rearrange("(t p q) -> p t q", p=128, q=QT),
                    )
                    mask_f.append(mf)

                # ---------- weight phase ----------
                # queryT[e, b] = tanh(sum_c Wc[c,e] c[b,c] + bias[e]), e on partitions.
                # Wc streamed in natural layout [c-part, e-free], rotating buffers.
                # Each matmul is single-shot (start+stop); kc-accumulation in SBUF.
                q_acc = const.tile([128, KD, BPC], fp32, tag="q_acc")
                for kc in range(KD):
                    wc_t = scratch.tile([128, D], fp32, tag="wstream", bufs=3, name="wc_t")
                    nc.sync.dma_start(out=wc_t, in_=wc_d[ts(kc, 128), :])
                    psum_qT = pp.tile(
                        [128, KD, BPC], fp32, tag="qT", bufs=1, name="psum_qT"
                    )
                    for ke in range(KD):
                        nc.tensor.matmul(
                            psum_qT[:, ke, :],
                            wc_t[:, ts(ke, 128)],
                            cT[:, :, kc],
                            start=True,
                            stop=True,
                        )
                    if kc == 0:
                        nc.vector.tensor_copy(q_acc, psum_qT)
                    else:
                        nc.vector.tensor_tensor(q_acc, q_acc, psum_qT, op=OP.add)
                nc.vector.tensor_tensor(q_acc, q_acc, biasT, op=OP.add)
                queryT = const.tile([128, KD, BPC], fp32, tag="queryT")
                nc.scalar.activation(queryT, q_acc, AF.Tanh)

                # q2[b, d] = scale * sum_e query[b,e] W[d,e].
                # Stream W natural chunks [d-chunk, e], transpose 128x128 blocks on
                # PE, and immediately contract with queryT into q2_psum_b[d-chunk].
                q2_psum = [
                    pp.tile([1, D], fp32, tag=f"q2ps{b}", bufs=1, name=f"q2ps{b}")
                    for b in range(BPC)
                ]
                for kd in range(KD):
                    wn = scratch.tile([128, D], fp32, tag="wstream", bufs=3, name="wn")
                    nc.sync.dma_start(out=wn, in_=w_d[ts(kd, 128), :])
                    for ke in range(KD):
                        pt = pp.tile([128, 512], fp32, tag="pb", bufs=3, name="pt")
                        nc.tensor.transpose(pt[:, 0:128], wn[:, ts(ke, 128)], identity)
                        wtb = scratch.tile([128, 128], fp32, tag="wtb", bufs=10, name="wtb")
                        nc.scalar.copy(wtb, pt[:, 0:128])
                        for b in range(BPC):
                            nc.tensor.matmul(
                                q2_psum[b][:, ts(kd, 128)],
                                queryT[:, ke, b : b + 1],
                                wtb,
                                start=(ke == 0),
                                stop=(ke == KD - 1),
                            )

                # scale, then broadcast each batch row to all 128 partitions
                q2b = []
                for b in range(BPC):
                    qb = persist.tile([128, D], fp32, tag=f"q2b{b}")
                    nc.vector.tensor_scalar_mul(qb[0:1, :], q2_psum[b], scale_sb)
                    for h in range(2):
                        pbc = pp.tile([128, 512], fp32, tag="pb", bufs=3, name="pbc")
                        nc.tensor.matmul(
                            pbc,
                            ones1,
                            qb[0:1, ts(h, 512)],
                            start=True,
                            stop=True,
                        )
                        nc.scalar.copy(qb[:, ts(h, 512)], pbc)
                    q2b.append(qb)

                if phase == 1:
                    for b in range(BPC):
                        nc.sync.dma_start(out=out_d[b : b + 1, :], in_=q2b[b][0:1, :])

                # ---------- main streaming pass ----------
                out_sb = [
                    const.tile([1, D], fp32, tag=f"out_sb{b}", name=f"out_sb{b}")
                    for b in range(BPC if phase >= 5 else 0)
                ]

                for b in range(BPC if phase >= 2 else 0):
                    # x tiles for this batch
                    x_tiles = []
                    for t in range(TS):
                        xt = xp.tile([128, XF], fp32, tag="xt")
                        nc.sync.dma_start(
                            out=xt,
                            in_=x_d[b, ts(t, SBLK), :].rearrange(
                                "(p q) d -> p (q d)", p=128
                            ),
                        )
                        x_tiles.append(xt)

                    if phase == 2:  # x-dma only
                        nc.sync.dma_start(
                            out=out_d[b : b + 1, 0:16], in_=x_tiles[0][0:1, 0:16]
                        )
                        continue

                    # eij[p, t, q] = <x[s], q2[b]>  for s = 512t + 4p + q
                    eij = persist.tile([128, TS, QT], fp32, tag=f"eij{b}")
                    for t in range(TS):
                        for q in range(QT):
                            sc = scratch.tile([128, D], fp32, tag="ttr_out")
                            nc.vector.scalar_tensor_tensor(
                                out=sc,
                                in0=x_tiles[t][:, ts(q, D)],
                                scalar=1.0,
                                in1=q2b[b],
                                op0=OP.mult,
                                op1=OP.mult,
                                accum_out=eij[:, t, q : q + 1],
                            )

                    if phase == 3:
                        nc.sync.dma_start(
                            out=out_d[b : b + 1, 0:16],
                            in_=eij[0:1, :, :],
                        )
                        continue

                    # softmax (masked, unnormalized; normalization folded into out)
                    # cross-partition max: per-partition max -> PE transpose ->
                    # free-dim max -> negate -> PE ones-broadcast back to [128,1]
                    m1 = scratch.tile([128, 1], fp32, tag="m1")
                    nc.vector.reduce_max(m1, eij, axis=mybir.AxisListType.XY)
                    pmax = pp.tile([1, 128], fp32, tag="pb", bufs=3, name="pmax")
                    nc.tensor.transpose(pmax, m1, identity)
                    negmx = scratch.tile([1, 1], fp32, tag="negmx")
                    nc.vector.reduce_max(negmx, pmax, axis=mybir.AxisListType.X, negate=True)
                    pbm = pp.tile([128, 1], fp32, tag="pb", bufs=3, name="pbm")
                    nc.tensor.matmul(pbm, ones1, negmx, start=True, stop=True)
                    negm = scratch.tile([128, 1], fp32, tag="negm")
                    nc.scalar.copy(negm, pbm)
                    a_b = persist.tile([128, TS, QT], fp32, tag=f"a{b}")
                    nc.scalar.activation(a_b, eij, AF.Exp, bias=negm, scale=1.0)
                    nc.vector.tensor_tensor(a_b, a_b, mask_f[b], op=OP.mult)

                    # cross-partition sum via PE ones-matmul
                    s1 = scratch.tile([128, 1], fp32, tag="s1")
                    nc.vector.reduce_sum(s1, a_b, axis=mybir.AxisListType.XY)
                    ssum = pp.tile([1, 1], fp32, tag="pb", bufs=3, name="ssum")
                    nc.tensor.matmul(ssum, s1, ones_col, start=True, stop=True)
                    den = scratch.tile([1, 1], fp32, tag="den")
                    nc.vector.tensor_scalar_add(den, ssum, EPS)
                    rden = scratch.tile([1, 1], fp32, tag="rden")
                    nc.vector.reciprocal(rden, den)

                    if phase == 4:
                        nc.sync.dma_start(
                            out=out_d[b : b + 1, 0:16],
                            in_=a_b[0:1, :, :],
                        )
                        continue

                    # out[b, d] = rden * sum_s a[s] x[s, d]
                    for h in range(2):
                        po = pp.tile([1, 512], fp32, tag="pb", bufs=3, name="po")
                        n = 0
                        for t in range(TS):
                            for q in range(QT):
                                nc.tensor.matmul(
                                    po,
                                    a_b[:, t, q : q + 1],
                                    x_tiles[t][:, q * D + h * 512 : q * D + (h + 1) * 512],
                                    start=(n == 0),
                                    stop=(n == TS * QT - 1),
                                )
                                n += 1
                        nc.vector.tensor_scalar_mul(
                            out_sb[b][:, ts(h, 512)], po, rden
                        )

                for b in range(BPC if phase >= 5 else 0):
                    nc.sync.dma_start(out=out_d[b : b + 1, :], in_=out_sb[b])

    nc.compile()
    return nc


def _get_nc():
    if "nc" not in _CACHE:
        _CACHE["nc"] = _build()
    return _CACHE["nc"]


def run(inputs, trace=False):
    from concourse.bass_utils import run_bass_kernel_spmd

    x = np.ascontiguousarray(inputs["x"], dtype=np.float32)
    mask = np.ascontiguousarray(inputs["mask"], dtype=np.int32)
    c = np.ascontiguousarray(inputs["c"], dtype=np.float32)
    W = np.ascontiguousarray(inputs["W"], dtype=np.float32)
    Wc = np.ascontiguousarray(inputs["Wc"], dtype=np.float32)
    b = np.ascontiguousarray(inputs["b"], dtype=np.float32)
    scale = np.ascontiguousarray(inputs["scale"], dtype=np.float32)

    in_maps = []
    for i in range(NCORES):
        sl = slice(i * BPC, (i + 1) * BPC)
        in_maps.append(
            {
                "x": x[sl],
                "mask": mask[sl],
                "c": c[sl],
                "W": W,
                "Wc": Wc,
                "b": b,
                "scale": scale,
            }
        )

    nc = _get_nc()
    res = run_bass_kernel_spmd(
        nc, in_maps, core_ids=list(range(NCORES)), trace=trace
    )
    out = np.concatenate([res.results[i]["out"] for i in range(NCORES)], axis=0)
    return out.astype(np.float32), res


def kernel(**inputs):
    out, _ = run(inputs, trace=False)
    return out



# revision 7
# speedup vs baseline: 1.0227x; 1.0227x over previous
"""Trainium2 Bass kernel for ContextAttentionMaskLuong.

Reference computation (per batch b):
    keys  = x @ W                       [B,S,D]
    query = tanh(c @ Wc + b)            [B,D]
    eij   = scale * <query, keys_s>     [B,S]
    a     = exp(eij - max) * mask; a /= (sum(a) + 1e-7)
    out   = sum_s a[s] * x[s,:]         [B,D]

Key rewrite: eij[b,s] = <x[b,s,:], q2[b]> with q2[b] = scale * (W @ query[b]),
which removes the [B,S,D]x[D,D] matmul entirely. The kernel is then one
streaming pass over x (memory-bound).

Sharding: data-parallel over batch: 16 batches / 8 cores = 2 per core.
W/Wc/b/scale replicated.

Per-core schedule (all x tiles resident in SBUF):
  - weight DMAs first on the SP HWDGE FIFO, then the 8 x tiles
  - Wc phase: 64 matmuls accumulated in PSUM -> bias add -> tanh -> queryT
  - W phase: 64 PE transposes (4 per PSUM tile, 16 wide ACT copies) -> wT,
    then 16 N=512 matmuls (both batches as lhsT cols) -> q2 [2, D]
  - q2 rows scaled on ACT, broadcast to 128 partitions via gpsimd
  - per batch: eij via 16 DVE scalar_tensor_tensor (fused mult+reduce),
    masked softmax (PE cross-partition reductions), 32 PE pooling matmuls,
    1/(sum+eps) folded into the PSUM->SBUF normalize
  - single gathered output DMA [1, 2*D]
"""

import numpy as np

B, S, D = 16, 2048, 1024
NCORES = 8
BPC = B // NCORES  # batches per core
EPS = 1e-7

TS = 4  # x tiles per batch
QT = 4  # s-rows per partition per tile
XF = QT * D  # x tile free size (4096)
SBLK = S // TS  # s-block per tile (512)
KD = D // 128  # 8 chunks of 128 along d/e/c

_CACHE = {}


def _build():
    import concourse.bass as bass
    import concourse.mybir as mybir
    import concourse.tile as tile
    from concourse import bacc
    from concourse.masks import make_identity

    fp32 = mybir.dt.float32
    i32 = mybir.dt.int32
    AF = mybir.ActivationFunctionType
    OP = mybir.AluOpType
    ts = bass.ts

    nc = bacc.Bacc(None)

    x_d = nc.dram_tensor("x", [BPC, S, D], fp32, kind="ExternalInput")
    mask_d = nc.dram_tensor("mask", [BPC, S], i32, kind="ExternalInput")
    c_d = nc.dram_tensor("c", [BPC, D], fp32, kind="ExternalInput")
    w_d = nc.dram_tensor("W", [D, D], fp32, kind="ExternalInput")
    wc_d = nc.dram_tensor("Wc", [D, D], fp32, kind="ExternalInput")
    b_d = nc.dram_tensor("b", [D], fp32, kind="ExternalInput")
    scale_d = nc.dram_tensor("scale", [1], fp32, kind="ExternalInput")
    out_d = nc.dram_tensor("out", [BPC, D], fp32, kind="ExternalOutput")

    with tile.TileContext(nc) as tc:
        with (
            tc.tile_pool(name="const", bufs=1) as const,
            tc.tile_pool(name="xp", bufs=7) as xp,
            tc.tile_pool(name="wst", bufs=2) as wst,
            tc.tile_pool(name="persist", bufs=1) as persist,
            tc.tile_pool(name="scratch", bufs=1) as scratch,
            tc.tile_pool(name="psum", bufs=1, space="PSUM") as pp,
        ):
            # ---------- constants / small loads ----------
            identity = const.tile([128, 128], fp32, tag="identity")
            make_identity(nc, identity)
            ones1 = const.tile([1, 128], fp32, tag="ones1")
            nc.vector.memset(ones1, 1.0)
            ones_col = const.tile([128, 1], fp32, tag="ones_col")
            nc.vector.memset(ones_col, 1.0)

            scale_sb = const.tile([2, 1], fp32, tag="scale")
            nc.sync.dma_start(out=scale_sb[0:1, :], in_=scale_d[None, :])
            nc.sync.dma_start(out=scale_sb[1:2, :], in_=scale_d[None, :])

            # bias with e on partitions: biasT[p, k] = b[128k+p]
            biasT = const.tile([128, KD], fp32, tag="biasT")
            nc.sync.dma_start(out=biasT, in_=b_d.rearrange("(k p) -> p k", p=128))

            # c transposed: cT[p, b, k] = c[b, 128k+p]  (single DMA)
            cT = const.tile([128, BPC, KD], fp32, tag="cT")
            nc.sync.dma_start(out=cT, in_=c_d.rearrange("b (k p) -> p b k", p=128))

            # mask (cast int32 -> f32 during DMA), layout matches eij
            mask_f = []
            for b in range(BPC):
                mf = persist.tile([128, TS, QT], fp32, tag=f"mask{b}")
                nc.gpsimd.dma_start(
                    out=mf,
                    in_=mask_d[b].rearrange("(t p q) -> p t q", p=128, q=QT),
                )
                mask_f.append(mf)

            # ---------- Wc phase: queryT[e-part, ke, b] ----------
            # single-shot matmuls per (kc, ke); kc-accumulation in SBUF
            # (interleaved PSUM accumulation groups in one bank are illegal)
            q_acc = const.tile([128, KD, BPC], fp32, tag="q_acc")
            for kc in range(KD):
                wc_t = wst.tile([128, D], fp32, tag="wstream", name="wc_t")
                nc.sync.dma_start(out=wc_t, in_=wc_d[ts(kc, 128), :])
                psq = pp.tile([128, KD, BPC], fp32, tag="psq", bufs=2, name="psq")
                for ke in range(KD):
                    nc.tensor.matmul(
                        psq[:, ke, :],
                        wc_t[:, ts(ke, 128)],
                        cT[:, :, kc],
                        start=True,
                        stop=True,
                    )
                if kc == 0:
                    nc.vector.tensor_copy(q_acc, psq)
                else:
                    nc.vector.tensor_tensor(q_acc, q_acc, psq, op=OP.add)
            q_biased = const.tile([128, KD, BPC], fp32, tag="q_biased")
            for b in range(BPC):
                nc.vector.tensor_tensor(
                    q_biased[:, :, b], q_acc[:, :, b], biasT, op=OP.add
                )
            queryT = const.tile([128, KD, BPC], fp32, tag="queryT")
            nc.scalar.activation(queryT, q_biased, AF.Tanh)

            # ---------- W phase: transpose W, then q2 = scale * (W @ query) ----
            # wT[p, ke, kd, j] = W[128*kd + j, 128*ke + p]
            wT = const.tile([128, KD, KD, 128], fp32, tag="wT")
            for kd in range(KD):
                wn = wst.tile([128, D], fp32, tag="wstream", name="wn")
                nc.sync.dma_start(out=wn, in_=w_d[ts(kd, 128), :])
                for g in range(2):
                    pt = pp.tile([128, 4, 128], fp32, tag="pt", bufs=2, name="pt")
                    for j in range(4):
                        ke = 4 * g + j
                        nc.tensor.transpose(
                            pt[:, j, :], wn[:, ts(ke, 128)], identity
                        )
                    nc.scalar.copy(wT[:, ts(g, 4), kd, :], pt)

            # q2 = scale * (W @ query): accumulate over ke per (b, half)
            qrows = []
            for b in range(BPC):
                qrow = persist.tile([1, D], fp32, tag=f"qrow{b}")
                for h in range(2):
                    q2ps = pp.tile([1, 512], fp32, tag="q2ps", bufs=2, name="q2ps")
                    for ke in range(KD):
                        nc.tensor.matmul(
                            q2ps,
                            queryT[:, ke, b : b + 1],
                            wT[:, ke, ts(h, 4), :],
                            start=(ke == 0),
                            stop=(ke == KD - 1),
                        )
                    nc.vector.tensor_scalar_mul(
                        qrow[:, ts(h, 512)], q2ps, scale_sb[0:1, :]
                    )
                qrows.append(qrow)

            # broadcast each q2 row to all 128 partitions
            q2b = []
            for b in range(BPC):
                qb = persist.tile([128, D], fp32, tag=f"q2b{b}")
                nc.gpsimd.partition_broadcast(qb, qrows[b])
                q2b.append(qb)

            # ---------- x DMAs (after the weight DMAs on the same FIFO) ------
            x_tiles = [[None] * TS for _ in range(BPC)]
            for b in range(BPC):
                for t in range(TS):
                    xt = xp.tile([128, XF], fp32, tag="xt")
                    nc.sync.dma_start(
                        out=xt,
                        in_=x_d[b, ts(t, SBLK), :].rearrange(
                            "(p q) d -> p (q d)", p=128
                        ),
                    )
                    x_tiles[b][t] = xt

            outrow = const.tile([1, BPC * D], fp32, tag="outrow")

            # ---------- main pass ----------
            for b in range(BPC):
                # eij[p, t, q] = <x[s], q2[b]>  for s = 512t + 4p + q
                eij = persist.tile([128, TS, QT], fp32, tag=f"eij{b}")
                for t in range(TS):
                    for q in range(QT):
                        sc = scratch.tile([128, D], fp32, tag="ttr_out")
                        nc.vector.scalar_tensor_tensor(
                            out=sc,
                            in0=x_tiles[b][t][:, ts(q, D)],
                            scalar=1.0,
                            in1=q2b[b],
                            op0=OP.mult,
                            op1=OP.mult,
                            accum_out=eij[:, t, q : q + 1],
                        )

                # masked softmax (unnormalized; normalization folded into out)
                m1 = scratch.tile([128, 1], fp32, tag="m1")
                nc.vector.reduce_max(m1, eij, axis=mybir.AxisListType.XY)
                pmax = pp.tile([1, 128], fp32, tag="pb", bufs=2, name="pmax")
                nc.tensor.transpose(pmax, m1, identity)
                negmx = scratch.tile([1, 1], fp32, tag="negmx")
                nc.vector.reduce_max(
                    negmx, pmax, axis=mybir.AxisListType.X, negate=True
                )
                pbm = pp.tile([128, 1], fp32, tag="pb", bufs=2, name="pbm")
                nc.tensor.matmul(pbm, ones1, negmx, start=True, stop=True)
                negm = scratch.tile([128, 1], fp32, tag="negm")
                nc.scalar.copy(negm, pbm)
                a_b = persist.tile([128, TS, QT], fp32, tag=f"a{b}")
                nc.scalar.activation(a_b, eij, AF.Exp, bias=negm, scale=1.0)
                nc.vector.tensor_tensor(a_b, a_b, mask_f[b], op=OP.mult)

                s1 = scratch.tile([128, 1], fp32, tag="s1")
                nc.vector.reduce_sum(s1, a_b, axis=mybir.AxisListType.XY)
                ssum = pp.tile([1, 1], fp32, tag="pb", bufs=2, name="ssum")
                nc.tensor.matmul(ssum, s1, ones_col, start=True, stop=True)
                den = scratch.tile([1, 1], fp32, tag="den")
                nc.vector.tensor_scalar_add(den, ssum, EPS)
                rden = scratch.tile([1, 1], fp32, tag="rden")
                nc.vector.reciprocal(rden, den)

                # out[b, d] = rden * sum_s a[s] x[s, d]
                for h in range(2):
                    po = pp.tile([1, 512], fp32, tag="pb", bufs=2, name="po")
                    n = 0
                    for t in range(TS):
                        for q in range(QT):
                            nc.tensor.matmul(
                                po,
                                a_b[:, t, q : q + 1],
                                x_tiles[b][t][
                                    :, q * D + h * 512 : q * D + (h + 1) * 512
                                ],
                                start=(n == 0),
                                stop=(n == TS * QT - 1),
                            )
                            n += 1
                    nc.vector.tensor_scalar_mul(
                        outrow[:, b * D + h * 512 : b * D + (h + 1) * 512],
                        po,
                        rden,
                    )

            nc.sync.dma_start(
                out=out_d.rearrange("b d -> (b d)")[None, :], in_=outrow
            )

    nc.compile()
    return nc


def _get_nc():
    if "nc" not in _CACHE:
        _CACHE["nc"] = _build()
    return _CACHE["nc"]


def run(inputs, trace=False):
    from concourse.bass_utils import run_bass_kernel_spmd

    x = np.ascontiguousarray(inputs["x"], dtype=np.float32)
    mask = np.ascontiguousarray(inputs["mask"], dtype=np.int32)
    c = np.ascontiguousarray(inputs["c"], dtype=np.float32)
    W = np.ascontiguousarray(inputs["W"], dtype=np.float32)
    Wc = np.ascontiguousarray(inputs["Wc"], dtype=np.float32)
    b = np.ascontiguousarray(inputs["b"], dtype=np.float32)
    scale = np.ascontiguousarray(inputs["scale"], dtype=np.float32)

    in_maps = []
    for i in range(NCORES):
        sl = slice(i * BPC, (i + 1) * BPC)
        in_maps.append(
            {
                "x": x[sl],
                "mask": mask[sl],
                "c": c[sl],
                "W": W,
                "Wc": Wc,
                "b": b,
                "scale": scale,
            }
        )

    nc = _get_nc()
    res = run_bass_kernel_spmd(
        nc, in_maps, core_ids=list(range(NCORES)), trace=trace
    )
    out = np.concatenate([res.results[i]["out"] for i in range(NCORES)], axis=0)
    return out.astype(np.float32), res


def kernel(**inputs):
    out, _ = run(inputs, trace=False)
    return out


# revision 11
# speedup vs baseline: 1.3603x; 1.3301x over previous
"""Trainium2 Bass kernel for ContextAttentionMaskLuong.

Reference computation (per batch b):
    keys  = x @ W                       [B,S,D]
    query = tanh(c @ Wc + b)            [B,D]
    eij   = scale * <query, keys_s>     [B,S]
    a     = exp(eij - max) * mask; a /= (sum(a) + 1e-7)
    out   = sum_s a[s] * x[s,:]         [B,D]

Key rewrite: eij[b,s] = <x[b,s,:], q2[b]> with q2[b] = scale * (W @ query[b]),
which removes the [B,S,D]x[D,D] matmul entirely. The kernel is then one
streaming pass over x (memory-bound).

fp32 matmul on PE is a 2-pass emulation (fp32_mode=LOW_HIGH) — so all PE
operands are cast to bf16 during DMA (halves HBM traffic too); accumulation
stays fp32 in PSUM, and the softmax chain is fp32 throughout.

Sharding: data-parallel over batch: 16 batches / 8 cores = 2 per core.
W/Wc/b/scale replicated.

Per-core schedule (all x tiles resident in SBUF as bf16):
  - Wc phase: 64 bf16 matmuls (single-shot) + SBUF fp32 accumulate -> tanh
  - W phase: 64 bf16 PE transposes (4 per PSUM tile, 16 wide ACT copies) ->
    wT bf16, then 32 N=512 bf16 matmuls -> q2 [1, D] per batch (fp32 PSUM)
  - q2 rows scaled, cast bf16, broadcast to 128 partitions via gpsimd
  - per batch: eij via 16 DVE scalar_tensor_tensor (bf16 in, fp32 accum),
    masked softmax in fp32 (PE cross-partition reductions), 32 bf16 PE
    pooling matmuls, 1/(sum+eps) folded into the PSUM->SBUF normalize
  - single gathered output DMA [1, 2*D]
"""

import numpy as np

B, S, D = 16, 2048, 1024
NCORES = 8
BPC = B // NCORES  # batches per core
EPS = 1e-7

TS = 4  # x tiles per batch
QT = 4  # s-rows per partition per tile
XF = QT * D  # x tile free size (4096)
SBLK = S // TS  # s-block per tile (512)
KD = D // 128  # 8 chunks of 128 along d/e/c

_CACHE = {}


def _build():
    import concourse.bass as bass
    import concourse.mybir as mybir
    import concourse.tile as tile
    from concourse import bacc
    from concourse.masks import make_identity

    fp32 = mybir.dt.float32
    bf16 = mybir.dt.bfloat16
    i32 = mybir.dt.int32
    AF = mybir.ActivationFunctionType
    OP = mybir.AluOpType
    ts = bass.ts

    nc = bacc.Bacc(None)

    x_d = nc.dram_tensor("x", [BPC, S, D], fp32, kind="ExternalInput")
    mask_d = nc.dram_tensor("mask", [BPC, S], i32, kind="ExternalInput")
    c_d = nc.dram_tensor("c", [BPC, D], fp32, kind="ExternalInput")
    w_d = nc.dram_tensor("W", [D, D], fp32, kind="ExternalInput")
    wc_d = nc.dram_tensor("Wc", [D, D], fp32, kind="ExternalInput")
    b_d = nc.dram_tensor("b", [D], fp32, kind="ExternalInput")
    scale_d = nc.dram_tensor("scale", [1], fp32, kind="ExternalInput")
    out_d = nc.dram_tensor("out", [BPC, D], fp32, kind="ExternalOutput")

    with tile.TileContext(nc) as tc:
        with (
            tc.tile_pool(name="const", bufs=1) as const,
            tc.tile_pool(name="xp", bufs=2 * TS) as xp,
            tc.tile_pool(name="wst", bufs=2) as wst,
            tc.tile_pool(name="persist", bufs=1) as persist,
            tc.tile_pool(name="scratch", bufs=1) as scratch,
            tc.tile_pool(name="psum", bufs=1, space="PSUM") as pp,
        ):
            # ---------- constants / small loads ----------
            identity = const.tile([128, 128], bf16, tag="identity")
            make_identity(nc, identity)
            identity32 = const.tile([128, 128], fp32, tag="identity32")
            make_identity(nc, identity32)
            ones1 = const.tile([1, 128], fp32, tag="ones1")
            nc.vector.memset(ones1, 1.0)
            ones_col = const.tile([128, 1], fp32, tag="ones_col")
            nc.vector.memset(ones_col, 1.0)

            scale_sb = const.tile([1, 1], fp32, tag="scale")
            nc.sync.dma_start(out=scale_sb, in_=scale_d[None, :])

            # bias with e on partitions: biasT[p, k] = b[128k+p]
            biasT = const.tile([128, KD], fp32, tag="biasT")
            nc.sync.dma_start(out=biasT, in_=b_d.rearrange("(k p) -> p k", p=128))

            # c transposed + cast bf16: cT[p, b, k] = c[b, 128k+p]
            cT = const.tile([128, BPC, KD], bf16, tag="cT")
            nc.gpsimd.dma_start(
                out=cT, in_=c_d.rearrange("b (k p) -> p b k", p=128)
            )

            # mask (cast int32 -> f32 during DMA), layout matches eij
            mask_f = []
            for b in range(BPC):
                mf = persist.tile([128, TS, QT], fp32, tag=f"mask{b}")
                nc.gpsimd.dma_start(
                    out=mf,
                    in_=mask_d[b].rearrange("(t p q) -> p t q", p=128, q=QT),
                )
                mask_f.append(mf)

            # ---------- Wc phase: queryT[e-part, ke, b] ----------
            # single-shot matmuls per (kc, ke); kc-accumulation in SBUF
            q_acc = const.tile([128, KD, BPC], fp32, tag="q_acc")
            for kc in range(KD):
                wc_t = wst.tile([128, D], bf16, tag="wstream", name="wc_t")
                nc.gpsimd.dma_start(out=wc_t, in_=wc_d[ts(kc, 128), :])
                psq = pp.tile([128, KD, BPC], fp32, tag="psq", bufs=2, name="psq")
                for ke in range(KD):
                    nc.tensor.matmul(
                        psq[:, ke, :],
                        wc_t[:, ts(ke, 128)],
                        cT[:, :, kc],
                        start=True,
                        stop=True,
                    )
                if kc == 0:
                    nc.vector.tensor_copy(q_acc, psq)
                else:
                    nc.vector.tensor_tensor(q_acc, q_acc, psq, op=OP.add)
            q_biased = const.tile([128, KD, BPC], fp32, tag="q_biased")
            for b in range(BPC):
                nc.vector.tensor_tensor(
                    q_biased[:, :, b], q_acc[:, :, b], biasT, op=OP.add
                )
            queryT = const.tile([128, KD, BPC], bf16, tag="queryT")
            nc.scalar.activation(queryT, q_biased, AF.Tanh)

            # ---------- W phase: transpose W, then q2 = scale*(W @ query) ----
            # wT[p, ke, kd, j] = W[128*kd + j, 128*ke + p]   (bf16)
            wT = const.tile([128, KD, KD, 128], bf16, tag="wT")
            for kd in range(KD):
                wn = wst.tile([128, D], bf16, tag="wstream", name="wn")
                nc.gpsimd.dma_start(out=wn, in_=w_d[ts(kd, 128), :])
                for g in range(2):
                    pt = pp.tile([128, 4, 128], bf16, tag="pt", bufs=2, name="pt")
                    for j in range(4):
                        ke = 4 * g + j
                        nc.tensor.transpose(
                            pt[:, j, :], wn[:, ts(ke, 128)], identity
                        )
                    nc.scalar.copy(wT[:, ts(g, 4), kd, :], pt)

            # q2 = scale * (W @ query): accumulate over ke per (b, half)
            qrows = []
            for b in range(BPC):
                qrow = persist.tile([1, D], bf16, tag=f"qrow{b}")
                for h in range(2):
                    q2ps = pp.tile([1, 512], fp32, tag="q2ps", bufs=2, name="q2ps")
                    for ke in range(KD):
                        nc.tensor.matmul(
                            q2ps,
                            queryT[:, ke, b : b + 1],
                            wT[:, ke, ts(h, 4), :],
                            start=(ke == 0),
                            stop=(ke == KD - 1),
                        )
                    nc.vector.tensor_scalar_mul(
                        qrow[:, ts(h, 512)], q2ps, scale_sb
                    )
                qrows.append(qrow)

            # broadcast each q2 row to all 128 partitions
            q2b = []
            for b in range(BPC):
                qb = persist.tile([128, D], bf16, tag=f"q2b{b}")
                nc.gpsimd.partition_broadcast(qb, qrows[b])
                q2b.append(qb)

            # ---------- x DMAs (bf16 cast during DMA) ----------
            x_tiles = [[None] * TS for _ in range(BPC)]
            for b in range(BPC):
                for t in range(TS):
                    xt = xp.tile([128, XF], bf16, tag="xt")
                    nc.gpsimd.dma_start(
                        out=xt,
                        in_=x_d[b, ts(t, SBLK), :].rearrange(
                            "(p q) d -> p (q d)", p=128
                        ),
                    )
                    x_tiles[b][t] = xt

            outrow = const.tile([1, BPC * D], fp32, tag="outrow")

            # ---------- main pass ----------
            for b in range(BPC):
                # eij[p, t, q] = <x[s], q2[b]>  for s = 512t + 4p + q
                eij = persist.tile([128, TS, QT], fp32, tag=f"eij{b}")
                for t in range(TS):
                    for q in range(QT):
                        sc = scratch.tile([128, D], bf16, tag="ttr_out")
                        nc.vector.scalar_tensor_tensor(
                            out=sc,
                            in0=x_tiles[b][t][:, ts(q, D)],
                            scalar=1.0,
                            in1=q2b[b],
                            op0=OP.mult,
                            op1=OP.mult,
                            accum_out=eij[:, t, q : q + 1],
                        )

                # masked softmax (unnormalized; normalization folded into out)
                m1 = scratch.tile([128, 1], fp32, tag="m1")
                nc.vector.reduce_max(m1, eij, axis=mybir.AxisListType.XY)
                pmax = pp.tile([1, 128], fp32, tag="pb", bufs=2, name="pmax")
                nc.tensor.transpose(pmax, m1, identity32)
                negmx = scratch.tile([1, 1], fp32, tag="negmx")
                nc.vector.reduce_max(
                    negmx, pmax, axis=mybir.AxisListType.X, negate=True
                )
                pbm = pp.tile([128, 1], fp32, tag="pb", bufs=2, name="pbm")
                nc.tensor.matmul(pbm, ones1, negmx, start=True, stop=True)
                negm = scratch.tile([128, 1], fp32, tag="negm")
                nc.scalar.copy(negm, pbm)
                a_b = persist.tile([128, TS, QT], fp32, tag=f"a{b}")
                nc.scalar.activation(a_b, eij, AF.Exp, bias=negm, scale=1.0)
                nc.vector.tensor_tensor(a_b, a_b, mask_f[b], op=OP.mult)

                s1 = scratch.tile([128, 1], fp32, tag="s1")
                nc.vector.reduce_sum(s1, a_b, axis=mybir.AxisListType.XY)
                ssum = pp.tile([1, 1], fp32, tag="pb", bufs=2, name="ssum")
                nc.tensor.matmul(ssum, s1, ones_col, start=True, stop=True)
                den = scratch.tile([1, 1], fp32, tag="den")
                nc.vector.tensor_scalar_add(den, ssum, EPS)
                rden = scratch.tile([1, 1], fp32, tag="rden")
                nc.vector.reciprocal(rden, den)

                # bf16 copy of the softmax weights for the PE pooling
                a_bf = persist.tile([128, TS, QT], bf16, tag=f"abf{b}")
                nc.scalar.copy(a_bf, a_b)

                # out[b, d] = rden * sum_s a[s] x[s, d]
                for h in range(2):
                    po = pp.tile([1, 512], fp32, tag="pb", bufs=2, name="po")
                    n = 0
                    for t in range(TS):
                        for q in range(QT):
                            nc.tensor.matmul(
                                po,
                                a_bf[:, t, q : q + 1],
                                x_tiles[b][t][
                                    :, q * D + h * 512 : q * D + (h + 1) * 512
                                ],
                                start=(n == 0),
                                stop=(n == TS * QT - 1),
                            )
                            n += 1
                    nc.vector.tensor_scalar_mul(
                        outrow[:, b * D + h * 512 : b * D + (h + 1) * 512],
                        po,
                        rden,
                    )

            nc.sync.dma_start(
                out=out_d.rearrange("b d -> (b d)")[None, :], in_=outrow
            )

    nc.compile()
    return nc


def _get_nc():
    if "nc" not in _CACHE:
        _CACHE["nc"] = _build()
    return _CACHE["nc"]


def run(inputs, trace=False):
    from concourse.bass_utils import run_bass_kernel_spmd

    x = np.ascontiguousarray(inputs["x"], dtype=np.float32)
    mask = np.ascontiguousarray(inputs["mask"], dtype=np.int32)
    c = np.ascontiguousarray(inputs["c"], dtype=np.float32)
    W = np.ascontiguousarray(inputs["W"], dtype=np.float32)
    Wc = np.ascontiguousarray(inputs["Wc"], dtype=np.float32)
    b = np.ascontiguousarray(inputs["b"], dtype=np.float32)
    scale = np.ascontiguousarray(inputs["scale"], dtype=np.float32)

    in_maps = []
    for i in range(NCORES):
        sl = slice(i * BPC, (i + 1) * BPC)
        in_maps.append(
            {
                "x": x[sl],
                "mask": mask[sl],
                "c": c[sl],
                "W": W,
                "Wc": Wc,
                "b": b,
                "scale": scale,
            }
        )

    nc = _get_nc()
    res = run_bass_kernel_spmd(
        nc, in_maps, core_ids=list(range(NCORES)), trace=trace
    )
    out = np.concatenate([res.results[i]["out"] for i in range(NCORES)], axis=0)
    return out.astype(np.float32), res


def kernel(**inputs):
    out, _ = run(inputs, trace=False)
    return out
